# revision 45
# baseline (speedup 1.0000x reference)
"""Causal multi-head attention (B=2, S=2048, D=1024, H=16) on 8 trn2 cores.

Sharding: batch (2-way) x head-group (4-way) = 8 cores. Each core computes
QKV projection for its batch restricted to its 4 heads, causal attention,
and a row-parallel slice of the output projection; the host sums the 4
partial outputs per batch (the all-reduce of the row-parallel Wo matmul).

Per-core kernel (Tile framework, fp16 matmul operands / fp32 PSUM accum):
  - The host ships x pre-transposed ([D, S] fp16) and the weight slices in
    fp16, so contraction dims land on SBUF partitions with plain DMAs.
  - Q,K are produced in [feat, seq] layout (rhs = x^T), V in [seq, feat]
    layout (lhsT = x^T) with an extra ones-column per head so the PV matmul
    also produces the softmax denominator.
  - Scores are computed transposed, S_T[key, q] = K_blk.T @ Q. The two heads
    of a pair live on partitions 0:64 / 64:128, so their K=64 score matmuls
    map to distinct PE row-groups and run concurrently; emission interleaves
    j2-outer/head-inner to keep the pairs adjacent.
  - exp on ScalarE (scale folded in), one activation per (head, key-block
    pair) covering the full live column range; causal staircase masking via
    gpsimd.affine_select directly on the exp output (fully-masked column
    prefixes are never read by PV and stay garbage).
  - PV: out_T[65, q] = V_aug.T @ exp(S_T), accumulated over key blocks; row
    64 is the denominator. Normalization batches both heads of a pair: one
    [2,512] reciprocal, one K=2 fp32r broadcast matmul into [128,512].
  - Wo: out[q, :] = sum_c vw_T_c.T @ Wo_c; partials leave as fp16 via
    gpsimd PSUM->SBUF copies; host sums partials and adds bo in fp32.
"""

import numpy as np
import ml_dtypes
from contextlib import ExitStack

import concourse.bass as bass
import concourse.mybir as mybir
import concourse.tile as tile
from concourse import bacc
from concourse.bass_utils import run_bass_kernel_spmd

B, S, D, H, HD = 2, 2048, 1024, 16, 64
NCORES = 8
NHG = 4                  # head groups (cores per batch)
NH = H // NHG            # 4 local heads
FQK = NH * HD * 2        # 512 local q+k features
FV = NH * HD             # 256 local v features
QB = 512                 # query block (attention outer tile)
KB = 128                 # key block
NSC = S // QB            # 4 seq chunks
R32 = mybir.dt.float32r
F16 = mybir.dt.float16
F32 = mybir.dt.float32
F8 = mybir.dt.float8e4
DR = mybir.MatmulPerfMode.DoubleRow
EXP = mybir.ActivationFunctionType.Exp
GE = mybir.AluOpType.is_ge
# Wqk is pre-scaled by WSCALE host-side so fp8e4 quantization stays out of
# the subnormal range (raw std 0.002); folded back out in the exp scale.
WSCALE = 64.0


def _build_body(ctx, tc, x_d, x8_d, wqk8_d, wv_d, bqk_d, bv_d, wo_d, out_d):
    nc = tc.nc

    const = ctx.enter_context(tc.tile_pool(name="const", bufs=1))
    wq_pool = ctx.enter_context(tc.tile_pool(name="wqp", bufs=8))
    wvp = ctx.enter_context(tc.tile_pool(name="wvp", bufs=8))
    wop = ctx.enter_context(tc.tile_pool(name="wop", bufs=2))
    xt_pool = ctx.enter_context(tc.tile_pool(name="xtp", bufs=8))
    x8_pool = ctx.enter_context(tc.tile_pool(name="x8p", bufs=4))
    qk_pool = ctx.enter_context(tc.tile_pool(name="qkp", bufs=1))
    v_pool = ctx.enter_context(tc.tile_pool(name="vp", bufs=16))
    exp_pool = ctx.enter_context(tc.tile_pool(name="ep", bufs=6))
    vw_pool = ctx.enter_context(tc.tile_pool(name="vwp", bufs=2))
    rc_pool = ctx.enter_context(tc.tile_pool(name="rcp", bufs=3))
    os_pool = ctx.enter_context(tc.tile_pool(name="osp", bufs=6))
    p1 = ctx.enter_context(tc.tile_pool(name="p1", bufs=2, space="PSUM"))
    ps = ctx.enter_context(tc.tile_pool(name="ps", bufs=2, space="PSUM"))
    po = ctx.enter_context(tc.tile_pool(name="po", bufs=2, space="PSUM"))



    # ---- warmup ----
    # HAM clock-gate needs ~3.4us of sustained PE activity to reach 2.4 GHz,
    # and the first exp pays a ~2.7us ACT table load. Burn both during the
    # initial DMA wait with dummy work on a zero tile.
    warm = const.tile([128, QB], F16)
    nc.gpsimd.memset(warm, 0.0)
    ed = const.tile([128, 16], F16)
    nc.scalar.activation(ed, warm[:, 0:16], EXP, scale=1.0)
    for i in range(10):
        pd = p1.tile([128, QB], F32, name="pd", tag="p1")
        nc.tensor.matmul(pd, warm[:, 0:128], warm, start=True, stop=True)

    # ---- weights ----
    # bqk/bv first: tiny, and the bvb broadcast matmul is the first PE
    # instruction - queued behind the bulk weights it stalls the PE stream
    bqk_sb = const.tile([128, 4], F32)
    nc.sync.dma_start(bqk_sb, bqk_d.ap().rearrange("(f p) -> p f", p=128))
    bv_sb = const.tile([1, FV], F32)
    nc.sync.dma_start(bv_sb, bv_d.ap().rearrange("(o e) -> o e", o=1))
    # fp8 DoubleRow weights: [c2][p, j, f] holds Wqk[c2*256 + j*128 + p, f]
    wqk8_sb = []
    for c2 in range(4):
        t = wq_pool.tile([128, 2, FQK], F8, name=f"wqk{c2}", tag="wqk")
        nc.sync.dma_start(
            t, wqk8_d.ap()[c2 * 128:(c2 + 1) * 128, :].rearrange(
                "p (j f) -> p j f", j=2))
        wqk8_sb.append(t)
    # full-S x tiles, chunk-0 columns DMA'd first so the first projection
    # matmuls aren't gated on the whole tensor; big batched DMAs otherwise
    # (per-DMA issue cost on the sync queue is ~0.7us)
    x8f = []
    for c2 in range(4):
        xt = x8_pool.tile([128, 2, S], F8, name="x8", tag="x8")
        src = x8_d.ap()[c2 * 128:(c2 + 1) * 128, :].rearrange(
            "p (j s) -> p j s", j=2)
        nc.sync.dma_start(xt[:, :, 0:QB], src[:, :, 0:QB])
        x8f.append((xt, src))
    wv_sb = []
    for dc in range(8):
        t = wvp.tile([128, FV], F16, name=f"wv{dc}", tag="wv")
        nc.sync.dma_start(t, wv_d.ap()[dc * 128:(dc + 1) * 128, :])
        wv_sb.append(t)
    xTf = []
    for dc in range(8):
        xt = xt_pool.tile([128, S], F16, name="xt", tag="xt")
        nc.sync.dma_start(xt[:, 0:QB], x_d.ap()[dc * 128:(dc + 1) * 128, 0:QB])
        xTf.append(xt)
    wo_sb = []
    for c in range(2):
        t = wop.tile([128, D], F16, name=f"wo{c}", tag="wo")
        nc.sync.dma_start(t, wo_d.ap()[c * 128:(c + 1) * 128, :])
        wo_sb.append(t)
    for c2 in range(4):
        xt, src = x8f[c2]
        nc.sync.dma_start(xt[:, :, QB:S], src[:, :, QB:S])
    x8f = [t for t, _ in x8f]
    for dc in range(8):
        nc.sync.dma_start(xTf[dc][:, QB:S],
                          x_d.ap()[dc * 128:(dc + 1) * 128, QB:S])
    # v-bias broadcast across partitions on gpsimd (SBUF->SBUF)
    bvb_sb = const.tile([128, FV], F32)
    nc.gpsimd.partition_broadcast(bvb_sb, bv_sb)

    # ---- phase B: QKV projection ----
    qkT = [qk_pool.tile([128, S], F16, name=f"qkT{f}", tag=f"qkT{f}", bufs=1)
           for f in range(4)]
    v_tiles = []

    def make_B_groups(sc):
        # projection work of chunk sc as independently emittable groups, so
        # attention emission can inject them into the PE stream exactly where
        # the PE would otherwise idle waiting on ScalarE exp
        groups = []

        def qk_group(f, sc=sc):
            # Q,K in [feat, seq]: psum += Wqk_c2.T @ x8, fp8 DoubleRow (K=256)
            pq = p1.tile([128, QB], F32, name="pq", tag="p1")
            for c2 in range(4):
                nc.tensor.matmul(
                    pq, wqk8_sb[c2][:, :, f * 128:(f + 1) * 128],
                    x8f[c2][:, :, sc * QB:(sc + 1) * QB],
                    start=(c2 == 0), stop=(c2 == 3), perf_mode=DR)
            nc.vector.tensor_scalar_add(
                qkT[f][:, sc * QB:(sc + 1) * QB], pq, bqk_sb[:, f:f + 1])

        def v_group(sb, sc=sc):
            # V in [seq, feat]: psum += (x^T_blk).T @ Wv_chunk + ones column
            pv = p1.tile([128, FV], F32, name="pv", tag="p1")
            for dc in range(8):
                nc.tensor.matmul(
                    pv,
                    xTf[dc][:, sc * QB + sb * 128:sc * QB + (sb + 1) * 128],
                    wv_sb[dc], start=(dc == 0), stop=(dc == 7))
            vt = v_pool.tile([128, NH, HD + 1], F16, name="vt", tag="vt")
            nc.vector.tensor_add(vt[:, :, 0:HD],
                                 pv.rearrange("p (h e) -> p h e", h=NH),
                                 bvb_sb.rearrange("p (h e) -> p h e", h=NH))
            nc.gpsimd.memset(vt[:, :, HD:HD + 1], 1.0)
            v_tiles.append(vt)

        for f in range(4):
            groups.append(lambda f=f: qk_group(f))
        for sb in range(4):
            groups.append(lambda sb=sb: v_group(sb))
        return groups

    def emit_C(qi, pending):
        # pace the injected projection groups evenly across this chunk's
        # fill points so the PE never starves late in the chunk
        navail = len(pending)
        nslots = 4 * (qi + 1) + 2
        state = [0, 0]  # slots passed, groups popped

        def fill(n):
            state[0] += n
            want = min(navail, (state[0] * navail + nslots - 1) // nslots)
            while state[1] < want and pending:
                pending.popleft()()
                state[1] += 1

        # ---- attention + output projection for query chunk qi ----
        vwT = [vw_pool.tile([128, QB], F16, name=f"vwT{c}", tag=f"vwT{c}")
               for c in range(2)]
        for hp in range(2):
            pair = (2 * hp, 2 * hp + 1)
            nkb = (qi + 1) * 4
            poh, Q, Kt = {}, {}, {}
            for h in pair:
                poh[h] = po.tile([HD + 1, QB], F32, name="poh", tag="po")
                r0 = (h % 2) * 64
                Q[h] = qkT[h // 2][r0:r0 + 64, qi * QB:(qi + 1) * QB]
                Kt[h] = qkT[2 + h // 2][r0:r0 + 64, :]

            def koff(kb):
                # columns q < (kb - qi*4)*128 of a diagonal key-block are
                # fully masked: skip them in scores/exp/PV
                return max(0, (kb - qi * 4)) * KB

            for base in range(0, nkb, 2):
                diag = base >= qi * 4
                o0 = koff(base)
                psn = {h: ps.tile([128, 2 * QB], F32, name="psn", tag="ps")
                       for h in pair}
                # j2-outer / head-inner: adjacent matmuls hit distinct PE
                # row-groups (partitions 0:64 vs 64:128) and run concurrently
                for j2 in range(2):
                    kb = base + j2
                    off = koff(kb)
                    for h in pair:
                        nc.tensor.matmul(
                            psn[h][:, j2 * QB + off:(j2 + 1) * QB],
                            Kt[h][:, kb * KB:(kb + 1) * KB],
                            Q[h][:, off:QB], start=True, stop=True)
                fill(1)
                es = {}
                for h in pair:
                    e = exp_pool.tile([128, 2 * QB], F16, name="et", tag="et")
                    nc.scalar.activation(e[:, o0:2 * QB], psn[h][:, o0:2 * QB],
                                         EXP,
                                         scale=1.0 / (np.sqrt(HD) * WSCALE**2))
                    if diag:
                        # causal staircase: keep col q' >= partition k within
                        # each live [off:QB] slice (off == 128*j exactly)
                        for j2 in range(2):
                            off = koff(base + j2)
                            nc.gpsimd.affine_select(
                                out=e[:, j2 * QB + off:(j2 + 1) * QB],
                                in_=e[:, j2 * QB + off:(j2 + 1) * QB],
                                compare_op=GE, fill=0.0, base=0,
                                channel_multiplier=-1,
                                pattern=[[1, QB - off]])
                    es[h] = e
                for j2 in range(2):
                    kb = base + j2
                    off = koff(kb)
                    for h in pair:
                        nc.tensor.matmul(
                            poh[h][:, off:QB], v_tiles[kb][:, h, :],
                            es[h][:, j2 * QB + off:(j2 + 1) * QB],
                            start=(kb == 0), stop=(kb == nkb - 1))
            # normalization per head, pipelined: copy denominator row out of
            # PSUM, reciprocal, gpsimd partition-broadcast, scale the head
            fill(2)
            for i, h in enumerate(pair):
                sumh = rc_pool.tile([1, QB], F32, name="sumh", tag="sumh")
                nc.vector.tensor_copy(sumh, poh[h][HD:HD + 1, :])
                rch = rc_pool.tile([1, QB], F32, name="rch", tag="rch")
                nc.vector.reciprocal_approx_fast(rch, sumh)
                bcs = rc_pool.tile([64, QB], F32, name="bcs", tag="bcs")
                nc.gpsimd.partition_broadcast(bcs, rch)
                nc.vector.tensor_mul(vwT[hp][i * 64:(i + 1) * 64, :],
                                     poh[h][0:HD, :], bcs)
        for ql in range(4):
            osb = os_pool.tile([128, 2 * QB], F16, name="osb", tag="osb")
            for do in range(2):
                pw = p1.tile([128, QB], F32, name="pw", tag="p1")
                for c in range(2):
                    nc.tensor.matmul(pw, vwT[c][:, ql * 128:(ql + 1) * 128],
                                     wo_sb[c][:, do * QB:(do + 1) * QB],
                                     start=(c == 0), stop=(c == 1))
                # ScalarE is idle by the final chunk's output drain; share it
                if qi == NSC - 1 and do == 1:
                    nc.scalar.copy(osb[:, do * QB:(do + 1) * QB], pw)
                else:
                    nc.vector.tensor_copy(osb[:, do * QB:(do + 1) * QB], pw)
            nc.sync.dma_start(
                out_d.ap()[qi * QB + ql * 128: qi * QB + (ql + 1) * 128, :],
                osb)

    # All input DMAs issue upfront (pools hold all 4 chunks); chunk 0's
    # projection runs dense (HAM warmup), later chunks' projection groups are
    # injected into attention's PE-idle slots (exp waits). Each C(qi) flushes
    # every group of chunks <= qi+1 before C(qi+1) needs them.
    from collections import deque
    for g in make_B_groups(0):
        g()
    pending = deque()
    for qi in range(NSC):
        if qi + 1 < NSC:
            pending.extend(make_B_groups(qi + 1))
        emit_C(qi, pending)
        while pending:
            pending.popleft()()


_COMPILED = None


def get_compiled():
    global _COMPILED
    if _COMPILED is not None:
        return _COMPILED
    nc = bacc.Bacc("TRN2", target_bir_lowering=False, debug=False,
                   enable_asserts=False, num_devices=NCORES)
    x_d = nc.dram_tensor("x", [D, S], F16, kind="ExternalInput")
    x8_d = nc.dram_tensor("x8", [D // 2, 2 * S], F8, kind="ExternalInput")
    wqk8_d = nc.dram_tensor("wqk8", [D // 2, 2 * FQK], F8,
                            kind="ExternalInput")
    wv_d = nc.dram_tensor("wv", [D, FV], F16, kind="ExternalInput")
    bqk_d = nc.dram_tensor("bqk", [FQK], F32, kind="ExternalInput")
    bv_d = nc.dram_tensor("bv", [FV], F32, kind="ExternalInput")
    wo_d = nc.dram_tensor("wo", [FV, D], F16, kind="ExternalInput")
    out_d = nc.dram_tensor("out", [S, D], F16, kind="ExternalOutput")
    with tile.TileContext(nc) as tc:
        with ExitStack() as ctx:
            _build_body(ctx, tc, x_d, x8_d, wqk8_d, wv_d, bqk_d, bv_d, wo_d,
                        out_d)
    nc.compile()
    _COMPILED = nc
    return nc


def _pack_dr(a):
    """[D, cols] -> [D/2, 2*cols] fp8 with virtual row (p,j) of 256-chunk c2
    holding row c2*256 + j*128 + p (must match the kernel's rearrange)."""
    c = a.reshape(4, 2, 128, a.shape[1])
    c = np.ascontiguousarray(c.transpose(0, 2, 1, 3))
    return c.reshape(D // 2, 2 * a.shape[1]).astype(ml_dtypes.float8_e4m3)


def make_in_maps(x, Wqkv, bqkv, Wo):
    x = np.ascontiguousarray(np.asarray(x, dtype=np.float32))
    Wqkv = np.asarray(Wqkv, dtype=np.float32)
    bqkv = np.asarray(bqkv, dtype=np.float32)
    Wo = np.asarray(Wo, dtype=np.float32)
    in_maps = []
    xT = [np.ascontiguousarray(x[b].T) for b in range(B)]
    x8 = [_pack_dr(t) for t in xT]
    for c in range(NCORES):
        b, hg = divmod(c, NHG)
        qs = slice(hg * FV, (hg + 1) * FV)
        ks = slice(D + hg * FV, D + (hg + 1) * FV)
        vs = slice(2 * D + hg * FV, 2 * D + (hg + 1) * FV)
        wqk = np.concatenate([Wqkv[:, qs], Wqkv[:, ks]], axis=1)
        in_maps.append({
            "x": xT[b].astype(np.float16),
            "x8": x8[b],
            "wqk8": _pack_dr(wqk * WSCALE),
            "wv": np.ascontiguousarray(Wqkv[:, vs]).astype(np.float16),
            "bqk": np.ascontiguousarray(
                np.concatenate([bqkv[qs], bqkv[ks]])) * WSCALE,
            "bv": np.ascontiguousarray(bqkv[vs]),
            "wo": np.ascontiguousarray(Wo[hg * FV:(hg + 1) * FV, :]).astype(np.float16),
        })
    return in_maps


def run_sharded(x, Wqkv, bqkv, Wo, bo, **spmd_kwargs):
    nc = get_compiled()
    in_maps = make_in_maps(x, Wqkv, bqkv, Wo)
    res = run_bass_kernel_spmd(nc, in_maps, core_ids=list(range(NCORES)),
                               **spmd_kwargs)
    out = np.zeros((B, S, D), np.float32)
    for c in range(NCORES):
        out[c // NHG] += res.results[c]["out"].astype(np.float32)
    out += np.asarray(bo, dtype=np.float32)
    return out, res


def kernel(x, mask, Wqkv, bqkv, Wo, bo):
    out, _ = run_sharded(x, Wqkv, bqkv, Wo, bo)
    return out


# revision 52
# speedup vs baseline: 1.0393x; 1.0393x over previous
"""Causal multi-head attention (B=2, S=2048, D=1024, H=16) on 8 trn2 cores.

Sharding: batch (2-way) x head-group (4-way) = 8 cores. Each core computes
QKV projection for its batch restricted to its 4 heads, causal attention,
and a row-parallel slice of the output projection; the host sums the 4
partial outputs per batch (the all-reduce of the row-parallel Wo matmul).

Per-core kernel (Tile framework, fp16 matmul operands / fp32 PSUM accum):
  - The host ships x pre-transposed ([D, S] fp16) and the weight slices in
    fp16, so contraction dims land on SBUF partitions with plain DMAs.
  - Q,K are produced in [feat, seq] layout (rhs = x^T), V in [seq, feat]
    layout (lhsT = x^T) with an extra ones-column per head so the PV matmul
    also produces the softmax denominator.
  - Scores are computed transposed, S_T[key, q] = K_blk.T @ Q. The two heads
    of a pair live on partitions 0:64 / 64:128, so their K=64 score matmuls
    map to distinct PE row-groups and run concurrently; emission interleaves
    j2-outer/head-inner to keep the pairs adjacent.
  - exp on ScalarE (scale folded in), one activation per (head, key-block
    pair) covering the full live column range; causal staircase masking via
    gpsimd.affine_select directly on the exp output (fully-masked column
    prefixes are never read by PV and stay garbage).
  - PV: out_T[65, q] = V_aug.T @ exp(S_T), accumulated over key blocks; row
    64 is the denominator. Normalization batches both heads of a pair: one
    [2,512] reciprocal, one K=2 fp32r broadcast matmul into [128,512].
  - Wo: out[q, :] = sum_c vw_T_c.T @ Wo_c; partials leave as fp16 via
    gpsimd PSUM->SBUF copies; host sums partials and adds bo in fp32.
"""

import numpy as np
import ml_dtypes
from contextlib import ExitStack

import concourse.bass as bass
import concourse.mybir as mybir
import concourse.tile as tile
from concourse import bacc
from concourse.bass_utils import run_bass_kernel_spmd

B, S, D, H, HD = 2, 2048, 1024, 16, 64
NCORES = 8
NHG = 4                  # head groups (cores per batch)
NH = H // NHG            # 4 local heads
FQK = NH * HD * 2        # 512 local q+k features
FV = NH * HD             # 256 local v features
QB = 512                 # query block (attention outer tile)
KB = 128                 # key block
NSC = S // QB            # 4 seq chunks
R32 = mybir.dt.float32r
F16 = mybir.dt.float16
F32 = mybir.dt.float32
F8 = mybir.dt.float8e4
DR = mybir.MatmulPerfMode.DoubleRow
EXP = mybir.ActivationFunctionType.Exp
GE = mybir.AluOpType.is_ge
# Wqk is pre-scaled by WSCALE host-side so fp8e4 quantization stays out of
# the subnormal range (raw std 0.002); folded back out in the exp scale.
WSCALE = 64.0


def _build_body(ctx, tc, x_d, x8_d, wqk8_d, wv_d, bqk_d, bv_d, wo_d, out_d):
    nc = tc.nc

    const = ctx.enter_context(tc.tile_pool(name="const", bufs=1))
    xt_pool = ctx.enter_context(tc.tile_pool(name="xtp", bufs=1))
    x8_pool = ctx.enter_context(tc.tile_pool(name="x8p", bufs=1))
    qk_pool = ctx.enter_context(tc.tile_pool(name="qkp", bufs=1))
    v_pool = ctx.enter_context(tc.tile_pool(name="vp", bufs=16))
    exp_pool = ctx.enter_context(tc.tile_pool(name="ep", bufs=6))
    vw_pool = ctx.enter_context(tc.tile_pool(name="vwp", bufs=2))
    rc_pool = ctx.enter_context(tc.tile_pool(name="rcp", bufs=3))
    os_pool = ctx.enter_context(tc.tile_pool(name="osp", bufs=6))
    p1 = ctx.enter_context(tc.tile_pool(name="p1", bufs=2, space="PSUM"))
    ps = ctx.enter_context(tc.tile_pool(name="ps", bufs=2, space="PSUM"))
    po = ctx.enter_context(tc.tile_pool(name="po", bufs=2, space="PSUM"))



    # ---- warmup ----
    # HAM clock-gate needs ~3.4us of sustained PE activity to reach 2.4 GHz,
    # and the first exp pays a ~2.7us ACT table load. Burn both during the
    # initial DMA wait with dummy work on a zero tile.
    warm = const.tile([128, QB], F16)
    nc.gpsimd.memset(warm, 0.0)
    ed = const.tile([128, 16], F16)
    nc.scalar.activation(ed, warm[:, 0:16], EXP, scale=1.0)
    for i in range(10):
        pd = p1.tile([128, QB], F32, name="pd", tag="p1")
        nc.tensor.matmul(pd, warm[:, 0:128], warm, start=True, stop=True)

    # ---- weights ----
    # bqk/bv first: tiny, and the bvb broadcast matmul is the first PE
    # instruction - queued behind the bulk weights it stalls the PE stream
    bqk_sb = const.tile([128, 4], F32)
    nc.sync.dma_start(bqk_sb, bqk_d.ap().rearrange("(f p) -> p f", p=128))
    bv_sb = const.tile([1, FV], F32)
    nc.sync.dma_start(bv_sb, bv_d.ap().rearrange("(o e) -> o e", o=1))
    # All bulk inputs are host-packed with the 128-partition dim outermost so
    # each tensor loads in 1-2 big DMAs (per-DMA issue cost on the sync
    # queue is ~0.7us). Chunk-0 columns come first so the first projection
    # matmuls aren't gated on the whole tensor.
    # fp8 DoubleRow weights: [p, c2, j, f] holds Wqk[c2*256 + j*128 + p, f]
    wqk8_sb = const.tile([128, 4, 2, FQK], F8)
    nc.sync.dma_start(wqk8_sb,
                      wqk8_d.ap().rearrange("p (c j f) -> p c j f", c=4, j=2))
    x8f = x8_pool.tile([128, 4, 2, S], F8, name="x8", tag="x8")
    x8_src = x8_d.ap().rearrange("p (c j s) -> p c j s", c=4, j=2)
    nc.sync.dma_start(x8f[:, :, :, 0:QB], x8_src[:, :, :, 0:QB])
    wv_sb = const.tile([128, 8, FV], F16)
    nc.sync.dma_start(wv_sb, wv_d.ap().rearrange("p (d f) -> p d f", d=8))
    xTf = xt_pool.tile([128, 8, S], F16, name="xt", tag="xt")
    xT_src = x_d.ap().rearrange("p (d s) -> p d s", d=8)
    nc.sync.dma_start(xTf[:, :, 0:QB], xT_src[:, :, 0:QB])
    wo_sb = const.tile([128, 2, D], F16)
    nc.sync.dma_start(wo_sb, wo_d.ap().rearrange("p (c e) -> p c e", c=2))
    nc.sync.dma_start(x8f[:, :, :, QB:S], x8_src[:, :, :, QB:S])
    nc.sync.dma_start(xTf[:, :, QB:S], xT_src[:, :, QB:S])
    # v-bias broadcast across partitions on gpsimd (SBUF->SBUF)
    bvb_sb = const.tile([128, FV], F32)
    nc.gpsimd.partition_broadcast(bvb_sb, bv_sb)

    # ---- phase B: QKV projection ----
    qkT = [qk_pool.tile([128, S], F16, name=f"qkT{f}", tag=f"qkT{f}", bufs=1)
           for f in range(4)]
    v_tiles = []

    def make_B_groups(sc):
        # projection work of chunk sc as independently emittable groups, so
        # attention emission can inject them into the PE stream exactly where
        # the PE would otherwise idle waiting on ScalarE exp
        groups = []

        def qk_group(f, sc=sc):
            # Q,K in [feat, seq]: psum += Wqk_c2.T @ x8, fp8 DoubleRow (K=256)
            pq = p1.tile([128, QB], F32, name="pq", tag="p1")
            for c2 in range(4):
                nc.tensor.matmul(
                    pq, wqk8_sb[:, c2, :, f * 128:(f + 1) * 128],
                    x8f[:, c2, :, sc * QB:(sc + 1) * QB],
                    start=(c2 == 0), stop=(c2 == 3), perf_mode=DR)
            nc.vector.tensor_scalar_add(
                qkT[f][:, sc * QB:(sc + 1) * QB], pq, bqk_sb[:, f:f + 1])

        def v_group(sb, sc=sc):
            # V in [seq, feat]: psum += (x^T_blk).T @ Wv_chunk + ones column
            pv = p1.tile([128, FV], F32, name="pv", tag="p1")
            for dc in range(8):
                nc.tensor.matmul(
                    pv,
                    xTf[:, dc, sc * QB + sb * 128:sc * QB + (sb + 1) * 128],
                    wv_sb[:, dc, :], start=(dc == 0), stop=(dc == 7))
            vt = v_pool.tile([128, NH, HD + 1], F16, name="vt", tag="vt")
            nc.vector.tensor_add(vt[:, :, 0:HD],
                                 pv.rearrange("p (h e) -> p h e", h=NH),
                                 bvb_sb.rearrange("p (h e) -> p h e", h=NH))
            nc.vector.tensor_scalar(vt[:, :, HD:HD + 1], vt[:, :, 0:1], 0.0,
                                    1.0, op0=mybir.AluOpType.mult,
                                    op1=mybir.AluOpType.add)
            v_tiles.append(vt)

        for f in range(4):
            groups.append(lambda f=f: qk_group(f))
        for sb in range(4):
            groups.append(lambda sb=sb: v_group(sb))
        return groups

    def emit_C(qi, pending):
        # pace the injected projection groups evenly across this chunk's
        # fill points so the PE never starves late in the chunk
        navail = len(pending)
        nslots = 4 * (qi + 1) + 2
        state = [0, 0]  # slots passed, groups popped

        def fill(n):
            state[0] += n
            want = min(navail, (state[0] * navail + nslots - 1) // nslots)
            while state[1] < want and pending:
                pending.popleft()()
                state[1] += 1

        # ---- attention + output projection for query chunk qi ----
        vwT = [vw_pool.tile([128, QB], F16, name=f"vwT{c}", tag=f"vwT{c}")
               for c in range(2)]
        for hp in range(2):
            pair = (2 * hp, 2 * hp + 1)
            nkb = (qi + 1) * 4
            poh, Q, Kt = {}, {}, {}
            for h in pair:
                poh[h] = po.tile([HD + 1, QB], F32, name="poh", tag="po")
                r0 = (h % 2) * 64
                Q[h] = qkT[h // 2][r0:r0 + 64, qi * QB:(qi + 1) * QB]
                Kt[h] = qkT[2 + h // 2][r0:r0 + 64, :]

            def koff(kb):
                # columns q < (kb - qi*4)*128 of a diagonal key-block are
                # fully masked: skip them in scores/exp/PV
                return max(0, (kb - qi * 4)) * KB

            for base in range(0, nkb, 2):
                diag = base >= qi * 4
                o0 = koff(base)
                psn = {h: ps.tile([128, 2 * QB], F32, name="psn", tag="ps")
                       for h in pair}
                # j2-outer / head-inner: adjacent matmuls hit distinct PE
                # row-groups (partitions 0:64 vs 64:128) and run concurrently
                for j2 in range(2):
                    kb = base + j2
                    off = koff(kb)
                    for h in pair:
                        nc.tensor.matmul(
                            psn[h][:, j2 * QB + off:(j2 + 1) * QB],
                            Kt[h][:, kb * KB:(kb + 1) * KB],
                            Q[h][:, off:QB], start=True, stop=True)
                fill(1)
                es = {}
                for h in pair:
                    e = exp_pool.tile([128, 2 * QB], F16, name="et", tag="et")
                    nc.scalar.activation(e[:, o0:2 * QB], psn[h][:, o0:2 * QB],
                                         EXP,
                                         scale=1.0 / (np.sqrt(HD) * WSCALE**2))
                    if diag:
                        # causal staircase: keep col q' >= partition k within
                        # each live [off:QB] slice (off == 128*j exactly)
                        for j2 in range(2):
                            off = koff(base + j2)
                            nc.gpsimd.affine_select(
                                out=e[:, j2 * QB + off:(j2 + 1) * QB],
                                in_=e[:, j2 * QB + off:(j2 + 1) * QB],
                                compare_op=GE, fill=0.0, base=0,
                                channel_multiplier=-1,
                                pattern=[[1, QB - off]])
                    es[h] = e
                for j2 in range(2):
                    kb = base + j2
                    off = koff(kb)
                    for h in pair:
                        nc.tensor.matmul(
                            poh[h][:, off:QB], v_tiles[kb][:, h, :],
                            es[h][:, j2 * QB + off:(j2 + 1) * QB],
                            start=(kb == 0), stop=(kb == nkb - 1))
            # normalization per head, pipelined: copy denominator row out of
            # PSUM, reciprocal, gpsimd partition-broadcast, scale the head
            fill(2)
            for i, h in enumerate(pair):
                sumh = rc_pool.tile([1, QB], F32, name="sumh", tag="sumh")
                nc.vector.tensor_copy(sumh, poh[h][HD:HD + 1, :])
                rch = rc_pool.tile([1, QB], F32, name="rch", tag="rch")
                nc.vector.reciprocal_approx_fast(rch, sumh)
                bcs = rc_pool.tile([64, QB], F32, name="bcs", tag="bcs")
                nc.gpsimd.partition_broadcast(bcs, rch)
                nc.vector.tensor_mul(vwT[hp][i * 64:(i + 1) * 64, :],
                                     poh[h][0:HD, :], bcs)
        for ql in range(4):
            osb = os_pool.tile([128, 2 * QB], F16, name="osb", tag="osb")
            for do in range(2):
                pw = p1.tile([128, QB], F32, name="pw", tag="p1")
                for c in range(2):
                    nc.tensor.matmul(pw, vwT[c][:, ql * 128:(ql + 1) * 128],
                                     wo_sb[:, c, do * QB:(do + 1) * QB],
                                     start=(c == 0), stop=(c == 1))
                # ScalarE is idle by the final chunk's output drain; share it
                if qi == NSC - 1 and do == 1:
                    nc.scalar.copy(osb[:, do * QB:(do + 1) * QB], pw)
                else:
                    nc.vector.tensor_copy(osb[:, do * QB:(do + 1) * QB], pw)
            nc.sync.dma_start(
                out_d.ap()[qi * QB + ql * 128: qi * QB + (ql + 1) * 128, :],
                osb)

    # All input DMAs issue upfront (pools hold all 4 chunks); chunk 0's
    # projection runs dense (HAM warmup), later chunks' projection groups are
    # injected into attention's PE-idle slots (exp waits). Each C(qi) flushes
    # every group of chunks <= qi+1 before C(qi+1) needs them.
    from collections import deque
    for g in make_B_groups(0):
        g()
    pending = deque()
    for qi in range(NSC):
        if qi + 1 < NSC:
            pending.extend(make_B_groups(qi + 1))
        emit_C(qi, pending)
        while pending:
            pending.popleft()()


_COMPILED = None


def get_compiled():
    global _COMPILED
    if _COMPILED is not None:
        return _COMPILED
    nc = bacc.Bacc("TRN2", target_bir_lowering=False, debug=False,
                   enable_asserts=False, num_devices=NCORES)
    x_d = nc.dram_tensor("x", [128, 8 * S], F16, kind="ExternalInput")
    x8_d = nc.dram_tensor("x8", [128, 8 * S], F8, kind="ExternalInput")
    wqk8_d = nc.dram_tensor("wqk8", [128, 8 * FQK], F8, kind="ExternalInput")
    wv_d = nc.dram_tensor("wv", [128, 8 * FV], F16, kind="ExternalInput")
    bqk_d = nc.dram_tensor("bqk", [FQK], F32, kind="ExternalInput")
    bv_d = nc.dram_tensor("bv", [FV], F32, kind="ExternalInput")
    wo_d = nc.dram_tensor("wo", [128, 2 * D], F16, kind="ExternalInput")
    out_d = nc.dram_tensor("out", [S, D], F16, kind="ExternalOutput")
    with tile.TileContext(nc) as tc:
        with ExitStack() as ctx:
            _build_body(ctx, tc, x_d, x8_d, wqk8_d, wv_d, bqk_d, bv_d, wo_d,
                        out_d)
    nc.compile()
    _COMPILED = nc
    return nc


def _pack_dr(a):
    """[D, cols] -> [128, 8*cols] fp8, layout (p)(c2, j, cols) with virtual
    row (p,j) of chunk c2 holding row c2*256 + j*128 + p (must match the
    kernel's rearrange)."""
    c = a.reshape(4, 2, 128, a.shape[1])
    c = np.ascontiguousarray(c.transpose(2, 0, 1, 3))
    return c.reshape(128, 8 * a.shape[1]).astype(ml_dtypes.float8_e4m3)


def _pack_p(a, n):
    """[n*128, cols] -> [128, n*cols], layout (p)(chunk, cols)."""
    c = a.reshape(n, 128, a.shape[1])
    return np.ascontiguousarray(c.transpose(1, 0, 2)).reshape(
        128, n * a.shape[1])


def make_in_maps(x, Wqkv, bqkv, Wo):
    x = np.ascontiguousarray(np.asarray(x, dtype=np.float32))
    Wqkv = np.asarray(Wqkv, dtype=np.float32)
    bqkv = np.asarray(bqkv, dtype=np.float32)
    Wo = np.asarray(Wo, dtype=np.float32)
    in_maps = []
    xT = [np.ascontiguousarray(x[b].T) for b in range(B)]
    x8 = [_pack_dr(t) for t in xT]
    for c in range(NCORES):
        b, hg = divmod(c, NHG)
        qs = slice(hg * FV, (hg + 1) * FV)
        ks = slice(D + hg * FV, D + (hg + 1) * FV)
        vs = slice(2 * D + hg * FV, 2 * D + (hg + 1) * FV)
        wqk = np.concatenate([Wqkv[:, qs], Wqkv[:, ks]], axis=1)
        in_maps.append({
            "x": _pack_p(xT[b], 8).astype(np.float16),
            "x8": x8[b],
            "wqk8": _pack_dr(wqk * WSCALE),
            "wv": _pack_p(Wqkv[:, vs], 8).astype(np.float16),
            "bqk": np.ascontiguousarray(
                np.concatenate([bqkv[qs], bqkv[ks]])) * WSCALE,
            "bv": np.ascontiguousarray(bqkv[vs]),
            "wo": _pack_p(Wo[hg * FV:(hg + 1) * FV, :], 2).astype(np.float16),
        })
    return in_maps


def run_sharded(x, Wqkv, bqkv, Wo, bo, **spmd_kwargs):
    nc = get_compiled()
    in_maps = make_in_maps(x, Wqkv, bqkv, Wo)
    res = run_bass_kernel_spmd(nc, in_maps, core_ids=list(range(NCORES)),
                               **spmd_kwargs)
    out = np.zeros((B, S, D), np.float32)
    for c in range(NCORES):
        out[c // NHG] += res.results[c]["out"].astype(np.float32)
    out += np.asarray(bo, dtype=np.float32)
    return out, res


def kernel(x, mask, Wqkv, bqkv, Wo, bo):
    out, _ = run_sharded(x, Wqkv, bqkv, Wo, bo)
    return out
